# revision 16
# baseline (speedup 1.0000x reference)
"""Trainium2 Bass kernel for sparse multi-head edge attention (V3).

Computation (per the nn.Module):
    Q = Fa @ Wq.T, K = Fb @ Wk.T, V = Fb @ Wv.T   (H=8 heads x 32)
    per edge e: logit[e,h] = <Q[a_e,h,:], K[b_e,h,:]> / sqrt(32)
    segmented softmax over edges per query, out = Fa + (softmax(V)) @ Wproj.T

Strategy (8 NeuronCores, SPMD, query-sharded, no collectives):
  - Core m owns queries [m*6250, (m+1)*6250).  Edges are routed to the owner
    of their query, grouped by 128-query block, padded to 128-edge tiles.
  - NO gather anywhere: the host distributes per-edge Fb rows (edge-sharded
    input layout, FbET = Fb[b_e].T in slot order) and the kernel recomputes
    K|V per edge with one fused matmul pair per 128-edge tile
    (KVe = FbE @ [Wk.T|Wv.T]).  This trades 2x matmul FLOPs for zero GPSIMD
    descriptor generation and no DRAM KV table round-trip — the previous
    dma_gather design was bottlenecked by ~10ns/row descriptor costs.
  - Per-edge Q rows come from a one-hot selection matmul (selT.T @ Qblk)
    with host-precomputed one-hot SEL/SELT streams (also encodes padding:
    pad slots have all-zero one-hot columns so they contribute exactly 0).
  - Work is spread across all four compute engines and batched in groups of
    G=2 tiles so per-instruction overheads amortize:
      PE:     KV matmuls, Qe selection, [den|num] accumulation, projection
      Scalar: Qe PSUM->SBUF copy, exp()
      DVE:    Q*K product (reads K from PSUM), per-head logit reduction,
              exp-weighted V
  - Softmax without max-subtraction (|logit| <~ 10 for this operator family,
    fp32/fp16 exp is safe); den clamped like the reference.
"""

import math

import numpy as np

P = 128
H = 8
DH = 32
CDIM = 256
NA = 50000
NB = 50000
NCORES = 8
NAC = NA // NCORES          # 6250 queries per core
NBLK = (NAC + P - 1) // P   # 49 query blocks per core
NPADQ = NBLK * P            # 6272 padded queries per core
CHUNK = 2048
G = 2                       # tiles per op-batching group
SCALE = 1.0 / math.sqrt(DH)

F16 = np.float16
F32 = np.float32


def _ceil128(x):
    return (np.asarray(x) + P - 1) // P * P


def preprocess(Fa, Fb, a_idx, b_idx, Wq, Wk, Wv, Wproj):
    """Host-side sharding: returns (meta, shared_inputs, per_core_inputs)."""
    a_idx = np.asarray(a_idx).astype(np.int64)
    b_idx = np.asarray(b_idx).astype(np.int64)
    Fa = np.asarray(Fa, F32)
    Fb = np.asarray(Fb, F32)

    core = a_idx // NAC
    a_loc = a_idx - core * NAC
    blk = a_loc // P
    arel = a_loc % P

    cnt = np.bincount(core * NBLK + blk, minlength=NCORES * NBLK)
    cnt = cnt.reshape(NCORES, NBLK)
    CAP = np.maximum(_ceil128(cnt.max(axis=0)), P)
    coff = np.concatenate([[0], np.cumsum(CAP)])
    TOT = int(coff[-1])

    # rank of each edge within its (core, blk) group
    ne = a_idx.shape[0]
    gid = core * NBLK + blk
    order = np.argsort(gid, kind="stable")
    counts = np.bincount(gid, minlength=NCORES * NBLK)
    gstart = np.concatenate([[0], np.cumsum(counts)])[:-1]
    rank = np.empty(ne, np.int64)
    rank[order] = np.arange(ne) - gstart[gid[order]]
    slot = coff[blk] + rank

    Fb16 = Fb.astype(F16)
    wkvT = np.concatenate([Wk.T, Wv.T], axis=1)          # [256, 512]
    shared = {
        "WqT": Wq.T.astype(F16).copy(),
        # contraction-half layout [k=128, half=2, n=512]
        "WKVT": wkvT.reshape(2, P, 2 * CDIM).transpose(1, 0, 2).astype(F16).copy(),
        "WprojT": Wproj.T.astype(F16).copy(),
        "IDENT": np.eye(P, dtype=F32),
    }

    per_core = []
    for m in range(NCORES):
        msk = core == m
        sl = slot[msk]
        ar = arel[msk]
        fbe = np.zeros((TOT, CDIM), F16)
        fbe[sl] = Fb16[b_idx[msk]]
        selT = np.zeros((P, TOT), F16)
        selT[ar, sl] = 1.0
        sel = np.zeros((P, TOT), F16)
        sel[sl % P, (sl // P) * P + ar] = 1.0

        FaT = np.zeros((CDIM, NPADQ), F16)
        FaT[:, :NAC] = Fa[m * NAC:(m + 1) * NAC].T.astype(F16)
        Fa_res = np.zeros((NPADQ, CDIM), F32)
        Fa_res[:NAC] = Fa[m * NAC:(m + 1) * NAC]
        fbet2 = fbe.T.reshape(2, P, TOT).transpose(1, 0, 2)   # [128, 2, TOT]
        per_core.append({
            "FbET": np.ascontiguousarray(fbet2),
            "SELT": selT,
            "SEL": sel,
            "FaT": FaT,
            "FaRes": Fa_res,
        })

    meta = {"CAP": CAP.astype(int), "coff": coff.astype(int), "TOT": TOT}
    return meta, shared, per_core


def build_program(meta):
    import concourse.bacc as bacc
    import concourse.mybir as mybir
    from concourse.tile import TileContext

    dt = mybir.dt
    nc = bacc.Bacc("TRN2", target_bir_lowering=False, debug=False,
                   num_devices=NCORES)

    CAP, coff, TOT = meta["CAP"], meta["coff"], meta["TOT"]
    CMAX = int(CAP.max()) // P
    AluOp = mybir.AluOpType

    FbET_t = nc.dram_tensor("FbET", [P, 2, TOT], dt.float16, kind="ExternalInput")
    SELT_t = nc.dram_tensor("SELT", [P, TOT], dt.float16, kind="ExternalInput")
    SEL_t = nc.dram_tensor("SEL", [P, TOT], dt.float16, kind="ExternalInput")
    FaT_t = nc.dram_tensor("FaT", [CDIM, NPADQ], dt.float16, kind="ExternalInput")
    FaRes_t = nc.dram_tensor("FaRes", [NPADQ, CDIM], dt.float32, kind="ExternalInput")
    WqT_t = nc.dram_tensor("WqT", [CDIM, CDIM], dt.float16, kind="ExternalInput")
    WKVT_t = nc.dram_tensor("WKVT", [P, 2, 2 * CDIM], dt.float16, kind="ExternalInput")
    WprojT_t = nc.dram_tensor("WprojT", [CDIM, CDIM], dt.float16, kind="ExternalInput")
    IDENT_t = nc.dram_tensor("IDENT", [P, P], dt.float32, kind="ExternalInput")
    OUT_t = nc.dram_tensor("OUT", [NPADQ, CDIM], dt.float32, kind="ExternalOutput")

    with TileContext(nc) as tc:
        with tc.tile_pool(name="res", bufs=1) as rpool:
            wq = rpool.tile([P, 2, CDIM], dt.float16, tag="wq")
            wkv = rpool.tile([P, 2, 2 * CDIM], dt.float16, tag="wkv")
            wproj = rpool.tile([P, 2, CDIM], dt.float16, tag="wproj")
            ident = rpool.tile([P, P], dt.float32, tag="ident")
            nc.sync.dma_start(out=wq[:, 0, :], in_=WqT_t[0:P, :])
            nc.sync.dma_start(out=wq[:, 1, :], in_=WqT_t[P:2 * P, :])
            nc.sync.dma_start(out=wkv[:], in_=WKVT_t[:, :, :])
            nc.sync.dma_start(out=wproj[:, 0, :], in_=WprojT_t[0:P, :])
            nc.sync.dma_start(out=wproj[:, 1, :], in_=WprojT_t[P:2 * P, :])
            nc.sync.dma_start(out=ident[:], in_=IDENT_t[:, :])
            qres = rpool.tile([P, NBLK, CDIM], dt.float16, tag="qres")

            # ---- Phase A: build Q into SBUF ----
            with tc.tile_pool(name="bld", bufs=2) as bpool, \
                 tc.tile_pool(name="psA", bufs=4, space="PSUM") as psA:
                for c0 in range(0, NPADQ, CHUNK):
                    nsub = min(CHUNK, NPADQ - c0) // P
                    ft = bpool.tile([P, 2, CHUNK], dt.float16, tag="ft")
                    nc.sync.dma_start(out=ft[:, 0, :nsub * P],
                                      in_=FaT_t[0:P, c0:c0 + nsub * P])
                    nc.sync.dma_start(out=ft[:, 1, :nsub * P],
                                      in_=FaT_t[P:2 * P, c0:c0 + nsub * P])
                    for s in range(nsub):
                        ps = psA.tile([P, CDIM], dt.float32, tag="psA")
                        nc.tensor.matmul(ps[:], ft[:, 0, s * P:(s + 1) * P],
                                         wq[:, 0, :], start=True, stop=False)
                        nc.tensor.matmul(ps[:], ft[:, 1, s * P:(s + 1) * P],
                                         wq[:, 1, :], start=False, stop=True)
                        nc.scalar.copy(out=qres[:, c0 // P + s, :], in_=ps[:])

            # ---- Phase B: edge attention ----
            with tc.tile_pool(name="gat", bufs=4) as gpool, \
                 tc.tile_pool(name="wrk", bufs=8) as wpool, \
                 tc.tile_pool(name="fin", bufs=3) as fpool, \
                 tc.tile_pool(name="psKV", bufs=2, space="PSUM") as psKV, \
                 tc.tile_pool(name="psQE", bufs=1, space="PSUM") as psQE, \
                 tc.tile_pool(name="psDN", bufs=2, space="PSUM") as psDN, \
                 tc.tile_pool(name="psFIN", bufs=1, space="PSUM") as psFIN:
                # software-pipelined schedule: stage B (exp, exwv, accum,
                # finalize) of group i-1 is emitted after stage A (matmuls,
                # qe-copy, prod, reduce) of group i, so no engine queue ever
                # holds an op whose dependency chain crosses engines within
                # the same group (in-order engine queues would stall on it).
                groups = []
                for j in range(NBLK):
                    Cj = int(CAP[j]) // P
                    for g0 in range(0, Cj, G):
                        groups.append((j, g0, min(G, Cj - g0), g0 + G >= Cj))

                blk = {}        # j -> dict(dn, sel, fbet, selt)

                def emit_finalize(j, dn_ps):
                    fin = psFIN.tile([P, 2 * CDIM], dt.float32, tag="fin")
                    den = fpool.tile([P, H], dt.float32, tag="den")
                    nc.vector.tensor_scalar_max(out=den[:], in0=dn_ps[:, 0:H],
                                                scalar1=1e-30)
                    rec = fpool.tile([P, H], dt.float32, tag="rec")
                    nc.vector.reciprocal(out=rec[:], in_=den[:])
                    s_sb = fpool.tile([P, CDIM], dt.float32, tag="s_sb")
                    nc.vector.tensor_tensor(
                        out=s_sb[:], in0=dn_ps[:, H:H + CDIM],
                        in1=rec[:].unsqueeze(2).to_broadcast([P, H, DH]),
                        op=AluOp.mult)
                    nc.tensor.transpose(fin[:, 0:P], s_sb[:, 0:P], ident[:])
                    nc.tensor.transpose(fin[:, P:2 * P], s_sb[:, P:2 * P], ident[:])
                    st_sb = fpool.tile([P, 2, P], dt.float16, tag="st_sb")
                    nc.scalar.copy(out=st_sb[:], in_=fin[:, 0:2 * P])
                    nc.tensor.matmul(fin[:, CDIM:2 * CDIM], st_sb[:, 0, :],
                                     wproj[:, 0, :], start=True, stop=False)
                    nc.tensor.matmul(fin[:, CDIM:2 * CDIM], st_sb[:, 1, :],
                                     wproj[:, 1, :], start=False, stop=True)
                    fa_t = fpool.tile([P, CDIM], dt.float32, tag="fa_t")
                    nc.sync.dma_start(out=fa_t[:], in_=FaRes_t[j * P:(j + 1) * P, :])
                    res = fpool.tile([P, CDIM], dt.float32, tag="res")
                    nc.vector.tensor_tensor(out=res[:], in0=fin[:, CDIM:2 * CDIM],
                                            in1=fa_t[:], op=AluOp.add)
                    nc.sync.dma_start(out=OUT_t[j * P:(j + 1) * P, :], in_=res[:])

                def emit_B(pb):
                    j, g0, gn, last = pb["key"]
                    Cj = int(CAP[j]) // P
                    exwv, logits, kv_ps = pb["exwv"], pb["logits"], pb["kv"]
                    nc.scalar.activation(
                        out=exwv[:, :gn, 0:H], in_=logits[:, :gn, :],
                        func=mybir.ActivationFunctionType.Exp, scale=SCALE)
                    nc.vector.tensor_tensor(
                        out=exwv[:, :gn, H:H + CDIM],
                        in0=kv_ps[:, :gn, CDIM:2 * CDIM],
                        in1=exwv[:, :gn, 0:H].unsqueeze(3).to_broadcast(
                            [P, gn, H, DH]),
                        op=AluOp.mult)
                    st = blk[j]
                    for t in range(g0, g0 + gn):
                        nc.tensor.matmul(st["dn"][:],
                                         st["sel"][:, t * P:(t + 1) * P],
                                         exwv[:, t - g0, :],
                                         start=(t == 0), stop=(t == Cj - 1))
                    if last:
                        emit_finalize(j, st["dn"])
                        del blk[j]

                pend = None
                for (j, g0, gn, last) in groups:
                    if j not in blk:
                        Cj = int(CAP[j]) // P
                        c0 = int(coff[j])
                        fbet = gpool.tile([P, 2, CMAX * P], dt.float16, tag="fbet")
                        nc.sync.dma_start(out=fbet[:, :, :Cj * P],
                                          in_=FbET_t[:, :, c0:c0 + Cj * P])
                        selt = gpool.tile([P, CMAX * P], dt.float16, tag="selt")
                        nc.sync.dma_start(out=selt[:, :Cj * P],
                                          in_=SELT_t[:, c0:c0 + Cj * P])
                        sel = gpool.tile([P, CMAX * P], dt.float16, tag="sel")
                        nc.sync.dma_start(out=sel[:, :Cj * P],
                                          in_=SEL_t[:, c0:c0 + Cj * P])
                        dn_ps = psDN.tile([P, H + CDIM], dt.float32, tag="dn")
                        blk[j] = {"fbet": fbet, "selt": selt, "sel": sel,
                                  "dn": dn_ps}
                    st = blk[j]
                    kv_ps = psKV.tile([P, G, 2 * CDIM], dt.float32, tag="kv")
                    qe_ps = psQE.tile([P, G, CDIM], dt.float32, tag="qe")
                    for t in range(g0, g0 + gn):
                        u = t - g0
                        nc.tensor.matmul(kv_ps[:, u, :],
                                         st["fbet"][:, 0, t * P:(t + 1) * P],
                                         wkv[:, 0, :], start=True, stop=False)
                        nc.tensor.matmul(kv_ps[:, u, :],
                                         st["fbet"][:, 1, t * P:(t + 1) * P],
                                         wkv[:, 1, :], start=False, stop=True)
                        nc.tensor.matmul(qe_ps[:, u, :],
                                         st["selt"][:, t * P:(t + 1) * P],
                                         qres[:, j, :], start=True, stop=True)
                    qe_sb = wpool.tile([P, G, CDIM], dt.float16, tag="qe_sb")
                    nc.scalar.copy(out=qe_sb[:, :gn, :], in_=qe_ps[:, :gn, :])
                    k_sb = wpool.tile([P, G, CDIM], dt.float16, tag="k_sb")
                    nc.scalar.copy(out=k_sb[:, :gn, :], in_=kv_ps[:, :gn, 0:CDIM])
                    prod = wpool.tile([P, G, CDIM], dt.float16, tag="prod")
                    nc.vector.tensor_tensor(
                        out=prod[:, :gn, :], in0=qe_sb[:, :gn, :],
                        in1=k_sb[:, :gn, :], op=AluOp.mult)
                    logits = wpool.tile([P, G, H], dt.float32, tag="logits")
                    nc.vector.tensor_reduce(
                        out=logits[:, :gn, :],
                        in_=prod[:, :gn, :].rearrange("p g (h d) -> p g h d", d=DH),
                        axis=mybir.AxisListType.X, op=AluOp.add)
                    exwv = wpool.tile([P, G, H + CDIM], dt.float16, tag="exwv")
                    if pend is not None:
                        emit_B(pend)
                    pend = {"key": (j, g0, gn, last), "exwv": exwv,
                            "logits": logits, "kv": kv_ps}
                if pend is not None:
                    emit_B(pend)

    nc.compile()
    return nc


TRACE = False          # set by test harness for NTFF profiling
LAST_RESULT = None     # BassKernelResults of the last run (for profiling)


def kernel(**inputs):
    global LAST_RESULT
    from concourse.bass_utils import run_bass_kernel_spmd

    meta, shared, per_core = preprocess(**inputs)
    nc = build_program(meta)
    in_maps = [dict(shared, **pc) for pc in per_core]
    res = run_bass_kernel_spmd(nc, in_maps, core_ids=list(range(NCORES)),
                               trace=TRACE)
    LAST_RESULT = res
    out = np.empty((NA, CDIM), F32)
    for m in range(NCORES):
        out[m * NAC:(m + 1) * NAC] = res.results[m]["OUT"][:NAC]
    return out


# revision 17
# speedup vs baseline: 1.3405x; 1.3405x over previous
"""Trainium2 Bass kernel for sparse multi-head edge attention (V3).

Computation (per the nn.Module):
    Q = Fa @ Wq.T, K = Fb @ Wk.T, V = Fb @ Wv.T   (H=8 heads x 32)
    per edge e: logit[e,h] = <Q[a_e,h,:], K[b_e,h,:]> / sqrt(32)
    segmented softmax over edges per query, out = Fa + (softmax(V)) @ Wproj.T

Strategy (8 NeuronCores, SPMD, query-sharded, no collectives):
  - Core m owns queries [m*6250, (m+1)*6250).  Edges are routed to the owner
    of their query, grouped by 128-query block, padded to 128-edge tiles.
  - NO gather anywhere: the host distributes per-edge Fb rows (edge-sharded
    input layout, FbET = Fb[b_e].T in slot order) and the kernel recomputes
    K|V per edge with one fused matmul pair per 128-edge tile
    (KVe = FbE @ [Wk.T|Wv.T]).  This trades 2x matmul FLOPs for zero GPSIMD
    descriptor generation and no DRAM KV table round-trip — the previous
    dma_gather design was bottlenecked by ~10ns/row descriptor costs.
  - Per-edge Q rows come from a one-hot selection matmul (selT.T @ Qblk)
    with host-precomputed one-hot SEL/SELT streams (also encodes padding:
    pad slots have all-zero one-hot columns so they contribute exactly 0).
  - Work is spread across all four compute engines and batched in groups of
    G=2 tiles so per-instruction overheads amortize:
      PE:     KV matmuls, Qe selection, [den|num] accumulation, projection
      Scalar: Qe PSUM->SBUF copy, exp()
      DVE:    Q*K product (reads K from PSUM), per-head logit reduction,
              exp-weighted V
  - Softmax without max-subtraction (|logit| <~ 10 for this operator family,
    fp32/fp16 exp is safe); den clamped like the reference.
"""

import math

import numpy as np

P = 128
H = 8
DH = 32
CDIM = 256
NA = 50000
NB = 50000
NCORES = 8
NAC = NA // NCORES          # 6250 queries per core
NBLK = (NAC + P - 1) // P   # 49 query blocks per core
NPADQ = NBLK * P            # 6272 padded queries per core
CHUNK = 2048
G = 2                       # tiles per op-batching group
SCALE = 1.0 / math.sqrt(DH)

F16 = np.float16
F32 = np.float32


def _ceil128(x):
    return (np.asarray(x) + P - 1) // P * P


def preprocess(Fa, Fb, a_idx, b_idx, Wq, Wk, Wv, Wproj):
    """Host-side sharding: returns (meta, shared_inputs, per_core_inputs)."""
    a_idx = np.asarray(a_idx).astype(np.int64)
    b_idx = np.asarray(b_idx).astype(np.int64)
    Fa = np.asarray(Fa, F32)
    Fb = np.asarray(Fb, F32)

    core = a_idx // NAC
    a_loc = a_idx - core * NAC
    blk = a_loc // P
    arel = a_loc % P

    cnt = np.bincount(core * NBLK + blk, minlength=NCORES * NBLK)
    cnt = cnt.reshape(NCORES, NBLK)
    CAP = np.maximum(_ceil128(cnt.max(axis=0)), P)
    coff = np.concatenate([[0], np.cumsum(CAP)])
    TOT = int(coff[-1])

    # rank of each edge within its (core, blk) group
    ne = a_idx.shape[0]
    gid = core * NBLK + blk
    order = np.argsort(gid, kind="stable")
    counts = np.bincount(gid, minlength=NCORES * NBLK)
    gstart = np.concatenate([[0], np.cumsum(counts)])[:-1]
    rank = np.empty(ne, np.int64)
    rank[order] = np.arange(ne) - gstart[gid[order]]
    slot = coff[blk] + rank

    Fb16 = Fb.astype(F16)
    wkvT = np.concatenate([Wk.T, Wv.T], axis=1)          # [256, 512]
    shared = {
        "WqT": Wq.T.astype(F16).copy(),
        # contraction-half layout [k=128, half=2, n=512]
        "WKVT": wkvT.reshape(2, P, 2 * CDIM).transpose(1, 0, 2).astype(F16).copy(),
        "WprojT": Wproj.T.astype(F16).copy(),
        "IDENT": np.eye(P, dtype=F32),
    }

    per_core = []
    for m in range(NCORES):
        msk = core == m
        sl = slot[msk]
        ar = arel[msk]
        fbe = np.zeros((TOT, CDIM), F16)
        fbe[sl] = Fb16[b_idx[msk]]
        selT = np.zeros((P, TOT), F16)
        selT[ar, sl] = 1.0
        sel = np.zeros((P, TOT), F16)
        sel[sl % P, (sl // P) * P + ar] = 1.0

        FaT = np.zeros((CDIM, NPADQ), F16)
        FaT[:, :NAC] = Fa[m * NAC:(m + 1) * NAC].T.astype(F16)
        Fa_res = np.zeros((NPADQ, CDIM), F32)
        Fa_res[:NAC] = Fa[m * NAC:(m + 1) * NAC]
        fbet2 = fbe.T.reshape(2, P, TOT).transpose(1, 0, 2)   # [128, 2, TOT]
        per_core.append({
            "FbET": np.ascontiguousarray(fbet2),
            "SELT": selT,
            "SEL": sel,
            "FaT": FaT,
            "FaRes": Fa_res,
        })

    meta = {"CAP": CAP.astype(int), "coff": coff.astype(int), "TOT": TOT}
    return meta, shared, per_core


def build_program(meta):
    import concourse.bacc as bacc
    import concourse.mybir as mybir
    from concourse.tile import TileContext

    dt = mybir.dt
    nc = bacc.Bacc("TRN2", target_bir_lowering=False, debug=False,
                   num_devices=NCORES)

    CAP, coff, TOT = meta["CAP"], meta["coff"], meta["TOT"]
    CMAX = int(CAP.max()) // P
    AluOp = mybir.AluOpType

    FbET_t = nc.dram_tensor("FbET", [P, 2, TOT], dt.float16, kind="ExternalInput")
    SELT_t = nc.dram_tensor("SELT", [P, TOT], dt.float16, kind="ExternalInput")
    SEL_t = nc.dram_tensor("SEL", [P, TOT], dt.float16, kind="ExternalInput")
    FaT_t = nc.dram_tensor("FaT", [CDIM, NPADQ], dt.float16, kind="ExternalInput")
    FaRes_t = nc.dram_tensor("FaRes", [NPADQ, CDIM], dt.float32, kind="ExternalInput")
    WqT_t = nc.dram_tensor("WqT", [CDIM, CDIM], dt.float16, kind="ExternalInput")
    WKVT_t = nc.dram_tensor("WKVT", [P, 2, 2 * CDIM], dt.float16, kind="ExternalInput")
    WprojT_t = nc.dram_tensor("WprojT", [CDIM, CDIM], dt.float16, kind="ExternalInput")
    IDENT_t = nc.dram_tensor("IDENT", [P, P], dt.float32, kind="ExternalInput")
    OUT_t = nc.dram_tensor("OUT", [NPADQ, CDIM], dt.float32, kind="ExternalOutput")

    with TileContext(nc) as tc:
        with tc.tile_pool(name="res", bufs=1) as rpool:
            wq = rpool.tile([P, 2, CDIM], dt.float16, tag="wq")
            wkv = rpool.tile([P, 2, 2 * CDIM], dt.float16, tag="wkv")
            wproj = rpool.tile([P, 2, CDIM], dt.float16, tag="wproj")
            ident = rpool.tile([P, P], dt.float32, tag="ident")
            nc.sync.dma_start(out=wq[:, 0, :], in_=WqT_t[0:P, :])
            nc.sync.dma_start(out=wq[:, 1, :], in_=WqT_t[P:2 * P, :])
            nc.sync.dma_start(out=wkv[:], in_=WKVT_t[:, :, :])
            nc.sync.dma_start(out=wproj[:, 0, :], in_=WprojT_t[0:P, :])
            nc.sync.dma_start(out=wproj[:, 1, :], in_=WprojT_t[P:2 * P, :])
            nc.sync.dma_start(out=ident[:], in_=IDENT_t[:, :])
            qres = rpool.tile([P, NBLK, CDIM], dt.float16, tag="qres")

            # ---- Phase A: build Q into SBUF ----
            with tc.tile_pool(name="bld", bufs=2) as bpool, \
                 tc.tile_pool(name="psA", bufs=4, space="PSUM") as psA:
                for c0 in range(0, NPADQ, CHUNK):
                    nsub = min(CHUNK, NPADQ - c0) // P
                    ft = bpool.tile([P, 2, CHUNK], dt.float16, tag="ft")
                    nc.sync.dma_start(out=ft[:, 0, :nsub * P],
                                      in_=FaT_t[0:P, c0:c0 + nsub * P])
                    nc.sync.dma_start(out=ft[:, 1, :nsub * P],
                                      in_=FaT_t[P:2 * P, c0:c0 + nsub * P])
                    for s in range(nsub):
                        ps = psA.tile([P, CDIM], dt.float32, tag="psA")
                        nc.tensor.matmul(ps[:], ft[:, 0, s * P:(s + 1) * P],
                                         wq[:, 0, :], start=True, stop=False)
                        nc.tensor.matmul(ps[:], ft[:, 1, s * P:(s + 1) * P],
                                         wq[:, 1, :], start=False, stop=True)
                        nc.scalar.copy(out=qres[:, c0 // P + s, :], in_=ps[:])

            # ---- Phase B: edge attention ----
            with tc.tile_pool(name="gat", bufs=4) as gpool, \
                 tc.tile_pool(name="wrk", bufs=8) as wpool, \
                 tc.tile_pool(name="fin", bufs=3) as fpool, \
                 tc.tile_pool(name="psKV", bufs=2, space="PSUM") as psKV, \
                 tc.tile_pool(name="psQE", bufs=1, space="PSUM") as psQE, \
                 tc.tile_pool(name="psDN", bufs=2, space="PSUM") as psDN, \
                 tc.tile_pool(name="psFIN", bufs=1, space="PSUM") as psFIN:
                # software-pipelined schedule: stage B (exp, exwv, accum,
                # finalize) of group i-1 is emitted after stage A (matmuls,
                # qe-copy, prod, reduce) of group i, so no engine queue ever
                # holds an op whose dependency chain crosses engines within
                # the same group (in-order engine queues would stall on it).
                groups = []
                for j in range(NBLK):
                    Cj = int(CAP[j]) // P
                    for g0 in range(0, Cj, G):
                        groups.append((j, g0, min(G, Cj - g0), g0 + G >= Cj))

                blk = {}        # j -> dict(dn, sel, fbet, selt)

                def emit_finalize(j, dn_ps):
                    fin = psFIN.tile([P, 2 * CDIM], dt.float32, tag="fin")
                    den = fpool.tile([P, H], dt.float32, tag="den")
                    nc.vector.tensor_scalar_max(out=den[:], in0=dn_ps[:, 0:H],
                                                scalar1=1e-30)
                    rec = fpool.tile([P, H], dt.float32, tag="rec")
                    nc.vector.reciprocal(out=rec[:], in_=den[:])
                    s_sb = fpool.tile([P, CDIM], dt.float32, tag="s_sb")
                    nc.vector.tensor_tensor(
                        out=s_sb[:], in0=dn_ps[:, H:H + CDIM],
                        in1=rec[:].unsqueeze(2).to_broadcast([P, H, DH]),
                        op=AluOp.mult)
                    nc.tensor.transpose(fin[:, 0:P], s_sb[:, 0:P], ident[:])
                    nc.tensor.transpose(fin[:, P:2 * P], s_sb[:, P:2 * P], ident[:])
                    st_sb = fpool.tile([P, 2, P], dt.float16, tag="st_sb")
                    nc.scalar.copy(out=st_sb[:], in_=fin[:, 0:2 * P])
                    nc.tensor.matmul(fin[:, CDIM:2 * CDIM], st_sb[:, 0, :],
                                     wproj[:, 0, :], start=True, stop=False)
                    nc.tensor.matmul(fin[:, CDIM:2 * CDIM], st_sb[:, 1, :],
                                     wproj[:, 1, :], start=False, stop=True)
                    fa_t = fpool.tile([P, CDIM], dt.float32, tag="fa_t")
                    nc.sync.dma_start(out=fa_t[:], in_=FaRes_t[j * P:(j + 1) * P, :])
                    res = fpool.tile([P, CDIM], dt.float32, tag="res")
                    nc.vector.tensor_tensor(out=res[:], in0=fin[:, CDIM:2 * CDIM],
                                            in1=fa_t[:], op=AluOp.add)
                    nc.sync.dma_start(out=OUT_t[j * P:(j + 1) * P, :], in_=res[:])

                def emit_B(pb):
                    j, g0, gn, last = pb["key"]
                    Cj = int(CAP[j]) // P
                    exwv, logits, kv_ps = pb["exwv"], pb["logits"], pb["kv"]
                    nc.scalar.activation(
                        out=exwv[:, :gn, 0:H], in_=logits[:, :gn, :],
                        func=mybir.ActivationFunctionType.Exp, scale=SCALE)
                    nc.vector.tensor_tensor(
                        out=exwv[:, :gn, H:H + CDIM],
                        in0=kv_ps[:, :gn, CDIM:2 * CDIM],
                        in1=exwv[:, :gn, 0:H].unsqueeze(3).to_broadcast(
                            [P, gn, H, DH]),
                        op=AluOp.mult)
                    st = blk[j]
                    for t in range(g0, g0 + gn):
                        nc.tensor.matmul(st["dn"][:],
                                         st["sel"][:, t * P:(t + 1) * P],
                                         exwv[:, t - g0, :],
                                         start=(t == 0), stop=(t == Cj - 1))
                    if last:
                        emit_finalize(j, st["dn"])
                        del blk[j]

                pend = None
                for (j, g0, gn, last) in groups:
                    if j not in blk:
                        Cj = int(CAP[j]) // P
                        c0 = int(coff[j])
                        fbet = gpool.tile([P, 2, CMAX * P], dt.float16, tag="fbet")
                        nc.sync.dma_start(out=fbet[:, :, :Cj * P],
                                          in_=FbET_t[:, :, c0:c0 + Cj * P])
                        selt = gpool.tile([P, CMAX * P], dt.float16, tag="selt")
                        nc.sync.dma_start(out=selt[:, :Cj * P],
                                          in_=SELT_t[:, c0:c0 + Cj * P])
                        sel = gpool.tile([P, CMAX * P], dt.float16, tag="sel")
                        nc.sync.dma_start(out=sel[:, :Cj * P],
                                          in_=SEL_t[:, c0:c0 + Cj * P])
                        dn_ps = psDN.tile([P, H + CDIM], dt.float32, tag="dn")
                        blk[j] = {"fbet": fbet, "selt": selt, "sel": sel,
                                  "dn": dn_ps}
                    st = blk[j]
                    kv_ps = psKV.tile([P, G, 2 * CDIM], dt.float32, tag="kv")
                    qe_ps = psQE.tile([P, G, CDIM], dt.float32, tag="qe")
                    for t in range(g0, g0 + gn):
                        u = t - g0
                        nc.tensor.matmul(kv_ps[:, u, :],
                                         st["fbet"][:, 0, t * P:(t + 1) * P],
                                         wkv[:, 0, :], start=True, stop=False)
                        nc.tensor.matmul(kv_ps[:, u, :],
                                         st["fbet"][:, 1, t * P:(t + 1) * P],
                                         wkv[:, 1, :], start=False, stop=True)
                        nc.tensor.matmul(qe_ps[:, u, :],
                                         st["selt"][:, t * P:(t + 1) * P],
                                         qres[:, j, :], start=True, stop=True)
                    qe_sb = wpool.tile([P, G, CDIM], dt.float16, tag="qe_sb")
                    nc.scalar.copy(out=qe_sb[:, :gn, :], in_=qe_ps[:, :gn, :])
                    prod = wpool.tile([P, G, CDIM], dt.float16, tag="prod")
                    nc.vector.tensor_tensor(
                        out=prod[:, :gn, :], in0=qe_sb[:, :gn, :],
                        in1=kv_ps[:, :gn, 0:CDIM], op=AluOp.mult)
                    logits = wpool.tile([P, G, H], dt.float32, tag="logits")
                    nc.vector.tensor_reduce(
                        out=logits[:, :gn, :],
                        in_=prod[:, :gn, :].rearrange("p g (h d) -> p g h d", d=DH),
                        axis=mybir.AxisListType.X, op=AluOp.add)
                    exwv = wpool.tile([P, G, H + CDIM], dt.float16, tag="exwv")
                    if pend is not None:
                        emit_B(pend)
                    pend = {"key": (j, g0, gn, last), "exwv": exwv,
                            "logits": logits, "kv": kv_ps}
                if pend is not None:
                    emit_B(pend)

    nc.compile()
    return nc


TRACE = False          # set by test harness for NTFF profiling
LAST_RESULT = None     # BassKernelResults of the last run (for profiling)


def kernel(**inputs):
    global LAST_RESULT
    from concourse.bass_utils import run_bass_kernel_spmd

    meta, shared, per_core = preprocess(**inputs)
    nc = build_program(meta)
    in_maps = [dict(shared, **pc) for pc in per_core]
    res = run_bass_kernel_spmd(nc, in_maps, core_ids=list(range(NCORES)),
                               trace=TRACE)
    LAST_RESULT = res
    out = np.empty((NA, CDIM), F32)
    for m in range(NCORES):
        out[m * NAC:(m + 1) * NAC] = res.results[m]["OUT"][:NAC]
    return out
